# revision 1
# baseline (speedup 1.0000x reference)
"""Trainium2 Bass kernel for multi-head causal self-attention.

Problem: X [4, 2048, 1024] fp32, Wq/Wk/Wv/Wo [1024, 1024], H=16 heads, HD=64.
reference: out = softmax_causal((X@Wq) (X@Wk)^T / 8) (X@Wv) merged @ Wo.

Sharding over 8 NeuronCores: core c handles batch b = c // 2 and head group
hg = c % 2 (8 heads each). Each core computes a partial [2048, 1024] output
(its heads' contribution through Wo's row shard); the host sums the two
partials per batch (the tensor-parallel all-reduce, done during unsharding).

Per-core dataflow (bf16 operands, fp32 PSUM accumulation):
  X^T  [1024, 2048]  bf16 via XBAR DMA-transpose straight from DRAM
  Q^T,K^T [512, 2048] = (Wq chunk).T @ X^T   (partition-chunk pc = head pair)
  V    [2048, 8, 72]  = X^T.T @ Wv, heads strided, col 64 = ones
  S^T  [128k, 512q] psum = K^T.T @ Q^T  (two heads row-packed in the PE
       array; fully-causal-masked leading columns skipped on PE)
  E^T  = exp(S^T/8 [+ diag mask]) on ACT -> bf16 SBUF; masked cols zeroed
       by GpSimd memset
  O'   [72, 512] psum = [V_h | 1 | pad].T @ E^T accumulated over k-chunks;
       row 64 = softmax denominators. Copied to SBUF immediately (frees the
       PSUM bank), then reciprocal -> gpsimd partition_broadcast -> DVE
       multiply writes normalized O^T.
  OUT  [128s, 512c] = O^T.T @ Wo chunk, accumulated over 4 o-chunks
"""

import sys

for _p in ("/opt/trn_rl_repo", "/root/.axon_site/_ro/trn_rl_repo"):
    if _p not in sys.path:
        sys.path.insert(0, _p)

import ml_dtypes
import numpy as np

import concourse.bass as bass
import concourse.mybir as mybir
import concourse.tile as tile
from concourse import bacc
from concourse.bass_utils import run_bass_kernel_spmd

F32 = mybir.dt.float32
BF16 = mybir.dt.bfloat16
EXPF = mybir.ActivationFunctionType.Exp

B, S, D, H = 4, 2048, 1024, 16
HD = D // H           # 64
HL = H // 2           # 8 heads per core
DL = HL * HD          # 512 local proj width
NEG = -30000.0        # causal mask additive value (exp underflows to 0)
VW = 72               # AV lhsT width: 64 V cols + ones col + 7 pad


def build_program(s=S, d=D, hl=HL):
    dl = hl * HD
    n_st = s // 128          # s-tiles (128 rows)
    n_dc = d // 128          # d-chunks (projection contraction)
    n_pc = dl // 128         # Q^T/K^T partition chunks (= head pairs)
    n_q = s // 512           # q-chunks
    n_k = s // 128           # k-chunks
    n_cc = d // 512          # out column chunks

    nc = bacc.Bacc("TRN2", target_bir_lowering=False, debug=False)

    X = nc.dram_tensor("X", [s, d], BF16, kind="ExternalInput")
    WQ = nc.dram_tensor("WQ", [d, dl], BF16, kind="ExternalInput")
    WK = nc.dram_tensor("WK", [d, dl], BF16, kind="ExternalInput")
    WV = nc.dram_tensor("WV", [d, dl], BF16, kind="ExternalInput")
    WO = nc.dram_tensor("WO", [dl, d], BF16, kind="ExternalInput")
    OUT = nc.dram_tensor("OUT", [s, d], F32, kind="ExternalOutput")

    with tile.TileContext(nc) as tc:
        with tc.tile_pool(name="persist", bufs=1) as persist:
            # diagonal causal mask block (keep where q >= k)
            cmask = persist.tile([128, 128], F32)
            nc.gpsimd.memset(cmask[:], 0.0)
            nc.gpsimd.affine_select(
                out=cmask[:], in_=cmask[:],
                compare_op=mybir.AluOpType.is_ge, fill=NEG,
                base=0, pattern=[[1, 128]], channel_multiplier=-1,
            )

            qt = [persist.tile([128, s], BF16, name=f"qt{i}") for i in range(n_pc)]
            kt = [persist.tile([128, s], BF16, name=f"kt{i}") for i in range(n_pc)]
            vt = [persist.tile([128, hl, VW], BF16, name=f"vt{i}") for i in range(n_st)]

            _late_cm = tc.tile_pool(name="late", bufs=1)
            late = _late_cm.__enter__()
            wo = late.tile([128, n_pc, d], BF16)
            ot = [late.tile([128, s], BF16, name=f"ot{i}") for i in range(n_pc)]

            # ---- X^T + projections (interleaved by sequence block) ----
            with (
                tc.tile_pool(name="xtp", bufs=1) as xtp,
                tc.tile_pool(name="wp", bufs=1) as wp,
                tc.tile_pool(name="pps", bufs=3, space="PSUM") as pps,
            ):
                xt = [xtp.tile([128, s], BF16, name=f"xt{i}") for i in range(n_dc)]
                wq = wp.tile([128, n_dc, dl], BF16, tag="wq", name="wq")
                wk = wp.tile([128, n_dc, dl], BF16, tag="wk", name="wk")
                wv = wp.tile([128, n_dc, dl], BF16, tag="wv", name="wv")
                # wq first: the opening projection needs it; the X^T
                # transposes already dominate the ramp
                nc.sync.dma_start(
                    wq[:], WQ.ap().rearrange("(c p) m -> p c m", p=128))
                for dc in range(n_dc):
                    nc.sync.dma_start(
                        xt[dc][:], X[:, dc * 128:(dc + 1) * 128], transpose=True)
                for wsb, wdram in ((wk, WK), (wv, WV)):
                    nc.sync.dma_start(
                        wsb[:], wdram.ap().rearrange("(c p) m -> p c m", p=128))
                nc.sync.dma_start(
                    wo[:], WO.ap().rearrange("(c p) m -> p c m", p=128))
                for nq in range(s // 512):
                    for w, dst in ((wq, qt), (wk, kt)):
                        for pc in range(n_pc):
                            ps = pps.tile([128, 512], F32, tag="ps")
                            for dc in range(n_dc):
                                nc.tensor.matmul(
                                    ps[:], w[:, dc, pc * 128:(pc + 1) * 128],
                                    xt[dc][:, nq * 512:(nq + 1) * 512],
                                    start=(dc == 0), stop=(dc == n_dc - 1))
                            nc.scalar.copy(
                                dst[pc][:, nq * 512:(nq + 1) * 512], ps[:])
                    for st in range(4 * nq, 4 * nq + 4):
                        ps = pps.tile([128, dl], F32, tag="ps")
                        for dc in range(n_dc):
                            nc.tensor.matmul(
                                ps[:], xt[dc][:, st * 128:(st + 1) * 128],
                                wv[:, dc, :],
                                start=(dc == 0), stop=(dc == n_dc - 1))
                        nc.vector.memset(vt[st][:], 1.0)
                        nc.vector.tensor_copy(
                            vt[st][:, :, 0:64],
                            ps[:].rearrange("p (h e) -> p h e", h=hl))

            # ---- attention + output projection ----
            with (
                tc.tile_pool(name="work", bufs=4) as work,
                tc.tile_pool(name="norm", bufs=3) as norm_pool,
                tc.tile_pool(name="aps", bufs=4, space="PSUM") as aps,
                tc.tile_pool(name="avps", bufs=2, space="PSUM") as avps,
                tc.tile_pool(name="ops", bufs=1, space="PSUM") as ops,
            ):
                for j in range(n_q):
                    js = slice(j * 512, (j + 1) * 512)
                    for pc in range(n_pc):
                        av = [avps.tile([VW, 512], F32, tag="av", name=f"av{j}_{pc}_0"),
                              avps.tile([VW, 512], F32, tag="av", name=f"av{j}_{pc}_1")]
                        n_i = min(4 * j + 4, n_k)
                        for i in range(n_i):
                            r = i - 4 * j
                            rs = max(r, 0) * 128   # fully-masked leading cols
                            for h in (0, 1):
                                hs = slice(64 * h, 64 * h + 64)
                                stp = aps.tile([128, 512], F32, tag="stp")
                                nc.tensor.matmul(
                                    stp[:, rs:512],
                                    kt[pc][hs, i * 128:(i + 1) * 128],
                                    qt[pc][hs, j * 512 + rs:(j + 1) * 512],
                                    start=True, stop=True,
                                    tile_position=(64 * h, 0))
                                et = work.tile([128, 512], BF16, tag="et", bufs=6)
                                if r >= 0:
                                    nc.vector.tensor_add(
                                        stp[:, rs:rs + 128], stp[:, rs:rs + 128],
                                        cmask[:])
                                    if rs:
                                        nc.gpsimd.memset(et[:, 0:rs], 0.0)
                                nc.scalar.activation(
                                    et[:, rs:512], stp[:, rs:512], EXPF, scale=0.125)
                                nc.tensor.matmul(
                                    av[h][:], vt[i][:, 2 * pc + h, :], et[:],
                                    start=(i == 0), stop=(i == n_i - 1))
                        orws = []
                        dgp = norm_pool.tile(
                            [2, 512], F32, tag="dg", bufs=4, name=f"dg{j}_{pc}")
                        for h in (0, 1):
                            # free the av bank quickly: copy O' + denominators
                            orw = norm_pool.tile(
                                [VW, 512], F32, tag="orw", bufs=4,
                                name=f"orw{j}_{pc}_{h}")
                            nc.scalar.copy(orw[:], av[h][:])
                            orws.append(orw)
                            nc.sync.dma_start(
                                dgp[h:h + 1, :], orw[64:65, :])
                        rgp = norm_pool.tile(
                            [2, 512], F32, tag="rg", bufs=4, name=f"rg{j}_{pc}")
                        nc.vector.reciprocal(rgp[:], dgp[:])
                        for h in (0, 1):
                            orw = orws[h]
                            if h == 0:
                                rsrc = rgp[0:1, :]
                            else:
                                rsb = norm_pool.tile(
                                    [1, 512], F32, tag="rsb", bufs=4,
                                    name=f"rsb{j}_{pc}")
                                nc.sync.dma_start(rsb[:], rgp[1:2, :])
                                rsrc = rsb[:]
                            bc = norm_pool.tile(
                                [128, 512], F32, tag="bc", bufs=4,
                                name=f"bc{j}_{pc}_{h}")
                            nc.gpsimd.partition_broadcast(bc[:], rsrc)
                            if h == 0:
                                nc.vector.tensor_mul(
                                    ot[pc][0:64, js], orw[0:64, :], bc[0:64, :])
                            else:
                                sc = norm_pool.tile(
                                    [64, 512], BF16, tag="sc", bufs=4,
                                    name=f"sc{j}_{pc}_{h}")
                                nc.vector.tensor_mul(
                                    sc[:], orw[0:64, :], bc[0:64, :])
                                nc.sync.dma_start(ot[pc][64:128, js], sc[:])

                    last_j = j == n_q - 1 and n_pc > 1
                    for st in range(4 * j, min(4 * j + 4, n_st)):
                        for cc in range(n_cc):
                            osb = work.tile([128, 512], F32, tag="osb", bufs=2)
                            if last_j:
                                # pairs 0..n-2 accumulate and stage to SBUF
                                # while the last pair's normalization is
                                # still in flight; final pair added after
                                ps = ops.tile([128, 512], F32, tag="outp", bufs=2)
                                for pc in range(n_pc - 1):
                                    nc.tensor.matmul(
                                        ps[:], ot[pc][:, st * 128:(st + 1) * 128],
                                        wo[:, pc, cc * 512:(cc + 1) * 512],
                                        start=(pc == 0), stop=(pc == n_pc - 2))
                                nc.vector.tensor_copy(osb[:], ps[:])
                                psb = ops.tile([128, 512], F32, tag="outp", bufs=2)
                                nc.tensor.matmul(
                                    psb[:], ot[n_pc - 1][:, st * 128:(st + 1) * 128],
                                    wo[:, n_pc - 1, cc * 512:(cc + 1) * 512],
                                    start=True, stop=True)
                                nc.vector.tensor_add(osb[:], osb[:], psb[:])
                            else:
                                ps = ops.tile([128, 512], F32, tag="outp", bufs=2)
                                for pc in range(n_pc):
                                    nc.tensor.matmul(
                                        ps[:], ot[pc][:, st * 128:(st + 1) * 128],
                                        wo[:, pc, cc * 512:(cc + 1) * 512],
                                        start=(pc == 0), stop=(pc == n_pc - 1))
                                nc.vector.tensor_copy(osb[:], ps[:])
                            nc.sync.dma_start(
                                OUT[st * 128:(st + 1) * 128,
                                    cc * 512:(cc + 1) * 512],
                                osb[:])

            _late_cm.__exit__(None, None, None)

    nc.compile()
    return nc


_NC_CACHE = {}


def _get_program():
    key = (S, D, HL)
    if key not in _NC_CACHE:
        _NC_CACHE[key] = build_program()
    return _NC_CACHE[key]


def _bf16(a):
    return np.ascontiguousarray(a.astype(ml_dtypes.bfloat16))


def make_in_maps(X, Wq, Wk, Wv, Wo):
    in_maps = []
    for c in range(8):
        b, hg = c // 2, c % 2
        cs = slice(hg * DL, hg * DL + DL)
        in_maps.append({
            "X": _bf16(X[b]),
            "WQ": _bf16(Wq[:, cs]),
            "WK": _bf16(Wk[:, cs]),
            "WV": _bf16(Wv[:, cs]),
            "WO": _bf16(Wo[cs, :]),
        })
    return in_maps


def gather_out(results):
    out = np.empty((B, S, D), dtype=np.float32)
    for b in range(B):
        out[b] = results[2 * b]["OUT"] + results[2 * b + 1]["OUT"]
    return out


def kernel(X, Wq, Wk, Wv, Wo):
    X = np.asarray(X, dtype=np.float32)
    Wq = np.asarray(Wq, dtype=np.float32)
    Wk = np.asarray(Wk, dtype=np.float32)
    Wv = np.asarray(Wv, dtype=np.float32)
    Wo = np.asarray(Wo, dtype=np.float32)

    nc = _get_program()
    in_maps = make_in_maps(X, Wq, Wk, Wv, Wo)
    res = run_bass_kernel_spmd(nc, in_maps, list(range(8)), trace=False)
    return gather_out(res.results)


if __name__ == "__main__":
    rng = np.random.default_rng(0)
    scale = 1.0 / np.sqrt(D)
    inputs = {
        "X": rng.standard_normal((B, S, D), dtype=np.float32),
        "Wq": rng.standard_normal((D, D), dtype=np.float32) * scale,
        "Wk": rng.standard_normal((D, D), dtype=np.float32) * scale,
        "Wv": rng.standard_normal((D, D), dtype=np.float32) * scale,
        "Wo": rng.standard_normal((D, D), dtype=np.float32) * scale,
    }
    out = kernel(**inputs)
    print("kernel output shape:", out.shape)



# revision 6
# speedup vs baseline: 1.3840x; 1.3840x over previous
"""Trainium2 Bass kernel for multi-head causal self-attention.

Problem: X [4, 2048, 1024] fp32, Wq/Wk/Wv/Wo [1024, 1024], H=16 heads, HD=64.
reference: out = softmax_causal((X@Wq) (X@Wk)^T / 8) (X@Wv) merged @ Wo.

Sharding over 8 NeuronCores: core c handles batch b = c // 2 and head group
hg = c % 2 (8 heads each). Each core computes a partial [2048, 1024] output
(its heads' contribution through Wo's row shard); the host sums the two
partials per batch (the tensor-parallel all-reduce, done during unsharding).

v2 design notes (vs the phase-separated baseline):
  * Projections are interleaved with attention at matmul granularity so the
    PE never idles long enough for the HAM clock gate to re-throttle, and
    the ACT engine's exp throughput (the real constraint of the attention
    inner loop) is overlapped with projection matmuls.
  * Scores for both heads of a pair go into one [128, 2, 512] fp32 PSUM
    tile (2 banks) so a single ACTIVATE handles exp for both heads
    (halves ACT instruction overhead).
  * Causal masking: one batched DVE add of a [128, 2, 128] -30000 triangle
    per diagonal k-block; fully-masked leading columns are simply never
    computed (scores, exp, and AV all operate on [rs:512]).
  * Normalization uses reciprocal_approx_fast (~5x faster than the
    microcoded reciprocal) + gpsimd partition_broadcast.
  * PSUM evacuation (AV accumulators -> SBUF) on DVE, not ACT.
  * dc-major first projection so the PE starts as soon as the first X^T
    transpose chunk lands; X^T DMA issues split across the two HWDGE
    queues (sync + act); exp table preloaded via a dummy activation.
"""

import sys

for _p in ("/opt/trn_rl_repo", "/root/.axon_site/_ro/trn_rl_repo"):
    if _p not in sys.path:
        sys.path.insert(0, _p)

import ml_dtypes
import numpy as np

import concourse.bass as bass
import concourse.mybir as mybir
import concourse.tile as tile
from concourse import bacc
from concourse.bass_utils import run_bass_kernel_spmd

F32 = mybir.dt.float32
BF16 = mybir.dt.bfloat16
EXPF = mybir.ActivationFunctionType.Exp

B, S, D, H = 4, 2048, 1024, 16
HD = D // H           # 64
HL = H // 2           # 8 heads per core
DL = HL * HD          # 512 local proj width
NEG = -30000.0        # causal mask additive value (exp underflows to 0)
VW = 65               # AV lhsT width: 64 V cols + ones col (denominator row)


def build_program(s=S, d=D, hl=HL):
    dl = hl * HD
    n_st = s // 128          # s-tiles (128 rows)
    n_dc = d // 128          # d-chunks (projection contraction)
    n_pc = dl // 128         # partition chunks (= head pairs)
    n_q = s // 512           # q-chunks
    n_cc = d // 512          # out column chunks

    nc = bacc.Bacc("TRN2", target_bir_lowering=False, debug=False)

    X = nc.dram_tensor("X", [s, d], BF16, kind="ExternalInput")
    WQ = nc.dram_tensor("WQ", [d, dl], BF16, kind="ExternalInput")
    WK = nc.dram_tensor("WK", [d, dl], BF16, kind="ExternalInput")
    WV = nc.dram_tensor("WV", [d, dl], BF16, kind="ExternalInput")
    WO = nc.dram_tensor("WO", [dl, d], BF16, kind="ExternalInput")
    OUT = nc.dram_tensor("OUT", [s, d], F32, kind="ExternalOutput")

    with tile.TileContext(nc) as tc:
        with tc.tile_pool(name="persist", bufs=1) as persist:
            # exp table preload: a tiny activation up front makes walrus put
            # the ACT_TABLE_LOAD during the DMA ramp instead of on the
            # critical path of the first real exp.
            scr = persist.tile([128, 8], F32)
            nc.vector.memset(scr[:], 0.0)
            scr2 = persist.tile([128, 8], F32)
            nc.scalar.activation(scr2[:], scr[:], EXPF, scale=1.0)

            # [128, 2, 128] additive causal mask for two stacked diagonal
            # blocks: 0 where q >= k else -30000.
            cmask = persist.tile([128, 2, 128], F32)
            nc.gpsimd.memset(cmask[:], 0.0)
            nc.gpsimd.affine_select(
                out=cmask[:], in_=cmask[:],
                compare_op=mybir.AluOpType.is_ge, fill=NEG,
                base=0, pattern=[[0, 2], [1, 128]], channel_multiplier=-1,
            )

            xt = [persist.tile([128, s], BF16, name=f"xt{i}") for i in range(n_dc)]
            wq = persist.tile([128, n_dc, dl], BF16, name="wq")
            wk = persist.tile([128, n_dc, dl], BF16, name="wk")
            wv = persist.tile([128, n_dc, dl], BF16, name="wv")
            wo = persist.tile([128, n_pc, d], BF16, name="wo")
            qt = [persist.tile([128, s], BF16, name=f"qt{i}") for i in range(n_pc)]
            kt = [persist.tile([128, s], BF16, name=f"kt{i}") for i in range(n_pc)]
            vt = [persist.tile([128, hl, VW], BF16, name=f"vt{i}")
                  for i in range(n_st)]
            ot = [persist.tile([128, s], BF16, name=f"ot{i}") for i in range(n_pc)]

            # DMA issue split across the two HWDGE queues so the X^T
            # transposes (the ramp gate) overlap.
            nc.sync.dma_start(wq[:], WQ.ap().rearrange("(c p) m -> p c m", p=128))
            for dc in range(n_dc):
                eng = nc.sync if dc % 2 == 0 else nc.scalar
                eng.dma_start(
                    xt[dc][:], X[:, dc * 128:(dc + 1) * 128], transpose=True)
            nc.sync.dma_start(wk[:], WK.ap().rearrange("(c p) m -> p c m", p=128))
            nc.sync.dma_start(wv[:], WV.ap().rearrange("(c p) m -> p c m", p=128))
            nc.scalar.dma_start(wo[:], WO.ap().rearrange("(c p) m -> p c m", p=128))

            # ---- prologue: Q/K projection for j=0 in dc-major order so the
            # PE starts on xt[0] without waiting for the whole transpose.
            with tc.tile_pool(name="prol", bufs=1, space="PSUM") as prol:
                qps = [prol.tile([128, 512], F32, name=f"qps{pc}")
                       for pc in range(n_pc)]
                kps = [prol.tile([128, 512], F32, name=f"kps{pc}")
                       for pc in range(n_pc)]
                for dc in range(n_dc):
                    for pc in range(n_pc):
                        nc.tensor.matmul(
                            qps[pc][:], wq[:, dc, pc * 128:(pc + 1) * 128],
                            xt[dc][:, 0:512],
                            start=(dc == 0), stop=(dc == n_dc - 1))
                        nc.tensor.matmul(
                            kps[pc][:], wk[:, dc, pc * 128:(pc + 1) * 128],
                            xt[dc][:, 0:512],
                            start=(dc == 0), stop=(dc == n_dc - 1))
                for pc in range(n_pc):
                    nc.vector.tensor_copy(qt[pc][:, 0:512], qps[pc][:])
                    nc.vector.tensor_copy(kt[pc][:, 0:512], kps[pc][:])

            with (
                tc.tile_pool(name="pp", bufs=2, space="PSUM") as pp,
                tc.tile_pool(name="sp", bufs=2, space="PSUM") as sp,
                tc.tile_pool(name="avp", bufs=2, space="PSUM") as avp,
                tc.tile_pool(name="work", bufs=3) as work,
                tc.tile_pool(name="norm", bufs=4) as normp,
            ):
                def proj_v(st):
                    ps = pp.tile([128, dl], F32, tag="pp")
                    for dc in range(n_dc):
                        nc.tensor.matmul(
                            ps[:], xt[dc][:, st * 128:(st + 1) * 128],
                            wv[:, dc, :],
                            start=(dc == 0), stop=(dc == n_dc - 1))
                    nc.vector.memset(vt[st][:, :, 64:65], 1.0)
                    nc.vector.tensor_copy(
                        vt[st][:, :, 0:64],
                        ps[:].rearrange("p (h e) -> p h e", h=hl))

                def proj_qk(w, dst, pc, j1):
                    js1 = slice(j1 * 512, (j1 + 1) * 512)
                    ps = pp.tile([128, 512], F32, tag="pp")
                    for dc in range(n_dc):
                        nc.tensor.matmul(
                            ps[:], w[:, dc, pc * 128:(pc + 1) * 128],
                            xt[dc][:, js1],
                            start=(dc == 0), stop=(dc == n_dc - 1))
                    nc.vector.tensor_copy(dst[pc][:, js1], ps[:])

                def out_proj(j, st, cc, pcs, add_to=None, staged=False):
                    """Partial output projection over head pairs `pcs`.
                    Returns the staged SBUF tile (caller DMAs or adds)."""
                    ps = pp.tile([128, 512], F32, tag="pp")
                    for n, pc in enumerate(pcs):
                        nc.tensor.matmul(
                            ps[:], ot[pc][:, st * 128:(st + 1) * 128],
                            wo[:, pc, cc * 512:(cc + 1) * 512],
                            start=(n == 0), stop=(n == len(pcs) - 1))
                    if add_to is None:
                        # the 8 last-chunk partials are all alive at once, so
                        # they get a dedicated 8-deep rotation (a 3-deep one
                        # FIFO-deadlocks DVE behind the final adds).
                        if staged:
                            osb = work.tile([128, 512], F32, tag="osbp",
                                            bufs=8, name=f"osbp{st}_{cc}")
                        else:
                            osb = work.tile([128, 512], F32, tag="osb",
                                            bufs=3, name=f"osb{st}_{cc}")
                        nc.vector.tensor_copy(osb[:], ps[:])
                        return osb
                    nc.vector.tensor_add(add_to[:], add_to[:], ps[:])
                    return add_to

                def dma_out(st, cc, osb):
                    nc.sync.dma_start(
                        OUT[st * 128:(st + 1) * 128, cc * 512:(cc + 1) * 512],
                        osb[:])

                proj_v_queue = list(range(4))  # vt tiles for j=0 built first
                for st in proj_v_queue:
                    proj_v(st)

                for j in range(n_q):
                    js = slice(j * 512, (j + 1) * 512)
                    osb_partial = {}  # (st, cc) -> staged partial for j == last
                    for pc in range(n_pc):
                        # filler units: always-ready projection work used to
                        # keep the PE busy while ACT exp gates the attention
                        # dependency chain.
                        filler = []
                        if j + 1 < n_q:
                            filler.append(
                                lambda pc=pc, j1=j + 1: proj_qk(wq, qt, pc, j1))
                            filler.append(
                                lambda pc=pc, j1=j + 1: proj_qk(wk, kt, pc, j1))
                            if pc == n_pc - 1:
                                for st in range(4 * (j + 1), 4 * (j + 2)):
                                    filler.append(lambda st=st: proj_v(st))
                        elif pc == n_pc - 1:
                            # last unit of the kernel: stage the partial
                            # output projection over pairs 0..n-2 while pair
                            # n-1 finishes its attention.
                            for st in range(4 * j, 4 * j + 4):
                                for cc in range(n_cc):
                                    def frag(st=st, cc=cc):
                                        osb_partial[(st, cc)] = out_proj(
                                            j, st, cc, list(range(n_pc - 1)),
                                            staged=True)
                                    filler.append(frag)

                        n_i = 4 * j + 4
                        every = max(1, n_i // max(1, len(filler)))
                        av = [avp.tile([VW, 512], F32, tag="av",
                                       name=f"av{j}_{pc}_{h}") for h in (0, 1)]
                        ets = {}

                        def emit_av(i):
                            r = i - 4 * j
                            rs = max(r, 0) * 128
                            et = ets.pop(i)
                            for h in (0, 1):
                                nc.tensor.matmul(
                                    av[h][:, rs:512], vt[i][:, 2 * pc + h, :],
                                    et[:, h, rs:512],
                                    start=(i == 0), stop=(i == n_i - 1))

                        for i in range(n_i):
                            r = i - 4 * j
                            rs = max(r, 0) * 128
                            stp = sp.tile([128, 2, 512], F32, tag="sp")
                            for h in (0, 1):
                                nc.tensor.matmul(
                                    stp[:, h, rs:512],
                                    kt[pc][64 * h:64 * h + 64,
                                           i * 128:(i + 1) * 128],
                                    qt[pc][64 * h:64 * h + 64,
                                           j * 512 + rs:(j + 1) * 512],
                                    start=True, stop=True,
                                    tile_position=(64 * h, 0))
                            if r >= 0:
                                nc.vector.tensor_add(
                                    stp[:, :, rs:rs + 128],
                                    stp[:, :, rs:rs + 128], cmask[:])
                            et = work.tile([128, 2, 512], BF16, tag="et",
                                           bufs=4)
                            nc.scalar.activation(
                                et[:, :, rs:512], stp[:, :, rs:512], EXPF,
                                scale=0.125)
                            ets[i] = et
                            if i >= 2:
                                emit_av(i - 2)
                            if filler and i % every == every - 1:
                                filler.pop(0)()
                        emit_av(n_i - 2)
                        emit_av(n_i - 1)
                        for fr in filler:
                            fr()

                        # normalization: denominators live in av row 64.
                        for h in (0, 1):
                            orw = normp.tile([VW, 512], F32, tag="orw",
                                             bufs=4, name=f"orw{j}_{pc}_{h}")
                            nc.vector.tensor_copy(orw[:], av[h][:])
                            dd = normp.tile([1, 512], F32, tag="dd", bufs=4,
                                            name=f"dd{j}_{pc}_{h}")
                            nc.sync.dma_start(dd[:], orw[64:65, :])
                            rr = normp.tile([1, 512], F32, tag="rr", bufs=4,
                                            name=f"rr{j}_{pc}_{h}")
                            nc.vector.reciprocal_approx_fast(rr[:], dd[:])
                            bc = normp.tile([64, 512], F32, tag="bc", bufs=4,
                                            name=f"bc{j}_{pc}_{h}")
                            nc.gpsimd.partition_broadcast(bc[:], rr[:])
                            if h == 0:
                                nc.vector.tensor_mul(
                                    ot[pc][0:64, js], orw[0:64, :], bc[:])
                            else:
                                sc = normp.tile([64, 512], BF16, tag="sc",
                                                bufs=4, name=f"sc{j}_{pc}")
                                nc.vector.tensor_mul(sc[:], orw[0:64, :], bc[:])
                                nc.sync.dma_start(ot[pc][64:128, js], sc[:])

                    # output projection for this q-chunk
                    if j < n_q - 1:
                        for st in range(4 * j, 4 * j + 4):
                            for cc in range(n_cc):
                                osb = out_proj(j, st, cc, list(range(n_pc)))
                                dma_out(st, cc, osb)
                    else:
                        for st in range(4 * j, 4 * j + 4):
                            for cc in range(n_cc):
                                osb = out_proj(j, st, cc, [n_pc - 1],
                                               add_to=osb_partial[(st, cc)])
                                dma_out(st, cc, osb)

    nc.compile()
    return nc


_NC_CACHE = {}


def _get_program():
    key = (S, D, HL)
    if key not in _NC_CACHE:
        _NC_CACHE[key] = build_program()
    return _NC_CACHE[key]


def _bf16(a):
    return np.ascontiguousarray(a.astype(ml_dtypes.bfloat16))


def make_in_maps(X, Wq, Wk, Wv, Wo):
    in_maps = []
    for c in range(8):
        b, hg = c // 2, c % 2
        cs = slice(hg * DL, hg * DL + DL)
        in_maps.append({
            "X": _bf16(X[b]),
            "WQ": _bf16(Wq[:, cs]),
            "WK": _bf16(Wk[:, cs]),
            "WV": _bf16(Wv[:, cs]),
            "WO": _bf16(Wo[cs, :]),
        })
    return in_maps


def gather_out(results):
    out = np.empty((B, S, D), dtype=np.float32)
    for b in range(B):
        out[b] = results[2 * b]["OUT"] + results[2 * b + 1]["OUT"]
    return out


def kernel(X, Wq, Wk, Wv, Wo):
    X = np.asarray(X, dtype=np.float32)
    Wq = np.asarray(Wq, dtype=np.float32)
    Wk = np.asarray(Wk, dtype=np.float32)
    Wv = np.asarray(Wv, dtype=np.float32)
    Wo = np.asarray(Wo, dtype=np.float32)

    nc = _get_program()
    in_maps = make_in_maps(X, Wq, Wk, Wv, Wo)
    res = run_bass_kernel_spmd(nc, in_maps, list(range(8)), trace=False)
    return gather_out(res.results)


if __name__ == "__main__":
    rng = np.random.default_rng(0)
    scale = 1.0 / np.sqrt(D)
    inputs = {
        "X": rng.standard_normal((B, S, D), dtype=np.float32),
        "Wq": rng.standard_normal((D, D), dtype=np.float32) * scale,
        "Wk": rng.standard_normal((D, D), dtype=np.float32) * scale,
        "Wv": rng.standard_normal((D, D), dtype=np.float32) * scale,
        "Wo": rng.standard_normal((D, D), dtype=np.float32) * scale,
    }
    out = kernel(**inputs)
    print("kernel output shape:", out.shape)
